# revision 34
# baseline (speedup 1.0000x reference)
"""Trainium2 Bass kernel: cosine-attention + positional-adjacency mix + BiLSTM + softmax classifier.

Model (per sample, reference semantics):
    Xn   = X / ||X||_row
    Xa   = (Xn Xn^T) @ A_D @ X          (A_D = row-normalized exp(-|i-j|/8), constant)
    h    = BiLSTM(Xa)                    (fwd + bwd, H=256)
    out  = softmax(h @ Wc + bc)

Strategy: data-parallel over batch (4 samples / core x 8 cores). All device
matmuls in bf16 with fp32 PSUM accumulation. The whole feedforward runs in
"transposed" layout so the LSTM gate math operates on 128-partition tiles:
    Xa^T = X^T @ (A_D^T @ (Xn Xn^T))     -- lhsT operands are natural-layout
    gx^T = Wx^T @ Xa^T  (+b)             -- [4H, T] per direction
LSTM recurrence keeps z^T tiles [128, 8*B]; gates are host-permuted to
[i, f, o, g] so ACT does one Sigmoid (i,f,o) + one Tanh (g) per step.
h is stored directly as bf16 in a (T+1)-slot ring ("hstore") whose slices are
the matmul moving operands of the next step -- no per-step transposes/copies.

Wall-clock-per-call optimizations (the metric is dominated by host<->device
transfer + per-call program-size-proportional overhead, NOT device execute
time, which measures as negligible):
  - X is shipped as globally-scaled int8 (8 MB instead of 32 MB f32). The
    scale cancels exactly in the cosine normalization and is folded into Wx
    on the host for the value path, so the device never sees it.
  - Weights are NOT replicated 8x: each core uploads a distinct 1/8 column
    shard of the packed weights (one "PK" blob per core, with the f32
    bias sections embedded via bitcast) and an on-device AllGather
    reconstructs the full set on every core.
  - The positional adjacency A_D is generated on device (iota/abs/exp).
  - The BiLSTM recurrence and the per-sample feedforward run inside tc.For_i
    hardware loops with ds() dynamic offsets: ~1k static instructions
    instead of ~24k, which cuts ~1s/call of per-call NEFF handling overhead.
  - Output is written as bf16 (3.2 MB) and widened to f32 on host.
  - The jax persistent compilation cache is enabled so repeat calls skip the
    XLA/neuronx recompile.
"""

import os
import numpy as np
import ml_dtypes

os.environ.setdefault("JAX_COMPILATION_CACHE_DIR", "/tmp/jaxcache")
try:
    import jax
    jax.config.update("jax_compilation_cache_dir",
                      os.environ["JAX_COMPILATION_CACHE_DIR"])
    jax.config.update("jax_persistent_cache_min_entry_size_bytes", -1)
    jax.config.update("jax_persistent_cache_min_compile_time_secs", 0)
except Exception:
    pass

import concourse.bass as bass
from concourse.bass import ds
import concourse.mybir as mybir
import concourse.bacc as bacc
import concourse.tile as tile
from concourse import bass_utils

F32 = mybir.dt.float32
BF16 = mybir.dt.bfloat16
AF = mybir.ActivationFunctionType
ALU = mybir.AluOpType
BF16NP = ml_dtypes.bfloat16

B_ALL, T_FULL, D, H, V = 32, 512, 512, 256, 96
SIGMA = 8.0
NCORES = 8
BL = B_ALL // NCORES          # samples per core
G4 = 4 * H                    # 1024 gate dims (permuted order i,f,o,g)
NM = G4 // 128                # 8 gate m-tiles
KD = D // 128                 # 4
KH = H // 128                 # 2

# packed-weight shard layout (columns per core): WXf | WXb | WHf | WHb | WCp
WXS = KD * G4 // NCORES       # 512
WHS = KH * G4 // NCORES       # 256
VP = 392                      # WC cols padded 388 -> 392 (divisible by 8)
WCS = VP // NCORES            # 49
SH = 2 * WXS + 2 * WHS + WCS  # 1585 shard cols
SHP = SH + 1                  # pad col so f32 sections sit at even offsets
PKC = SHP + 2 * (2 * NM) + 2 * (V + 1)  # + f32 BIAS/BCREP as raw bf16
USE_ALLGATHER = True
# Ship X as globally-scaled int8 (8 MB instead of 16 MB bf16). The global
# scale cancels exactly in the cosine normalization; for the value path it is
# folded into Wx on the host, so the device never sees it.
X_INT8 = True
I8 = mybir.dt.int8


def _host_stationary(q):
    """[R, C] -> [128, (R//128)*C]; k-th col-block = rows k*128:(k+1)*128."""
    r, c = q.shape
    return np.ascontiguousarray(
        q.reshape(r // 128, 128, c).transpose(1, 0, 2).reshape(128, (r // 128) * c)
    )


def _permute_gates(w):
    """Reorder last-dim gate blocks [i,f,g,o] -> [i,f,o,g]."""
    i, f, g, o = np.split(w, 4, axis=-1)
    return np.concatenate([i, f, o, g], axis=-1)


def build_program(t_param=T_FULL, n_devices=NCORES, bl=BL, reps=1,
                  use_allgather=USE_ALLGATHER, x_int8=X_INT8):
    T = t_param
    KT = T // 128
    nc = bacc.Bacc(
        "TRN2", target_bir_lowering=False, debug=False, enable_asserts=False,
        num_devices=n_devices,
    )

    x_in = nc.dram_tensor("XBF", [bl, T, D], I8 if x_int8 else BF16,
                          kind="ExternalInput")
    out_d = nc.dram_tensor("OUT", [bl, T, V + 1], BF16, kind="ExternalOutput")

    if use_allgather:
        pk_in = nc.dram_tensor("PK", [128, PKC], BF16, kind="ExternalInput")
        bias_in = pk_in[:, SHP:SHP + 4 * NM].bitcast(F32)
        bc_in = pk_in[:, SHP + 4 * NM:PKC].bitcast(F32)
        wbnc = nc.dram_tensor("WBNC", [128, SH], BF16)
        wall = nc.dram_tensor("WALL", [n_devices, 128, SH], BF16,
                              addr_space="Shared")
    else:
        bias_in = nc.dram_tensor("BIAS", [128, 2 * NM], F32,
                                 kind="ExternalInput")[:]
        bc_in = nc.dram_tensor("BCREP", [128, V + 1], F32,
                               kind="ExternalInput")[:]
        wx_in = {d: nc.dram_tensor(f"WX{d}", [128, KD * G4], BF16,
                                   kind="ExternalInput") for d in "fb"}
        wh_in = {d: nc.dram_tensor(f"WH{d}", [128, KH * G4], BF16,
                                   kind="ExternalInput") for d in "fb"}
        wc_in = nc.dram_tensor("WC", [128, VP], BF16, kind="ExternalInput")

    B8 = 2 * bl    # h-cols per hstore slot
    GB = NM * bl   # z free cols (8*B)

    from contextlib import ExitStack
    with tile.TileContext(nc) as tc:
        _rep = ExitStack()
        if reps > 1:
            _rep.enter_context(tc.For_i(0, reps, 1))
        with (
            tc.tile_pool(name="const", bufs=1) as cpool,
            tc.tile_pool(name="gates", bufs=1) as gpool,
            tc.tile_pool(name="state", bufs=1) as spool,
        ):
            if use_allgather:
                nc.sync.dma_start(wbnc[:], pk_in[:, :SH])
                nc.gpsimd.collective_compute(
                    "AllGather", ALU.bypass,
                    replica_groups=[list(range(n_devices))],
                    ins=[wbnc[:].opt()], outs=[wall[:].opt()])
                c0 = 0
                wsrc = {}
                for nm_, w_ in [("wxf", WXS), ("wxb", WXS), ("whf", WHS),
                                ("whb", WHS), ("wc", WCS)]:
                    wsrc[nm_] = wall[:, :, c0:c0 + w_].rearrange("r p c -> p r c")
                    c0 += w_
            wx_sb = {}
            wh_sb = {}
            for d in "fb":
                wx_sb[d] = cpool.tile([128, KD * G4], BF16, name=f"wx_{d}")
                nc.sync.dma_start(
                    wx_sb[d][:], wsrc[f"wx{d}"] if use_allgather else wx_in[d][:])
                wh_sb[d] = cpool.tile([128, KH * G4], BF16, name=f"wh_{d}")
                nc.sync.dma_start(
                    wh_sb[d][:], wsrc[f"wh{d}"] if use_allgather else wh_in[d][:])
            wc_sb = cpool.tile([128, VP], BF16)
            nc.sync.dma_start(wc_sb[:], wsrc["wc"] if use_allgather else wc_in[:])
            bias_sb = cpool.tile([128, 2 * NM], F32)
            nc.sync.dma_start(bias_sb[:], bias_in)
            bc_sb = cpool.tile([128, V + 1], F32)
            nc.sync.dma_start(bc_sb[:], bc_in)

            # A_D generated on device: ad_sb[p, k*T+c] = AD[k*128+p, c]
            ad_sb = cpool.tile([128, KT * T], BF16)
            with tc.tile_pool(name="adgen", bufs=2) as agp:
                for k in range(KT):
                    df = agp.tile([128, T], F32, tag="df")
                    nc.gpsimd.iota(df[:], pattern=[[1, T]], base=-(k * 128),
                                   channel_multiplier=-1,
                                   allow_small_or_imprecise_dtypes=True)
                    ab = agp.tile([128, T], F32, tag="ab")
                    nc.scalar.activation(ab[:], df[:], AF.Abs)
                    ex = agp.tile([128, T], F32, tag="ex")
                    ssum = agp.tile([128, 1], F32, tag="ssum")
                    nc.scalar.activation(ex[:], ab[:], AF.Exp,
                                         scale=-1.0 / SIGMA, accum_out=ssum[:])
                    rs = agp.tile([128, 1], F32, tag="rs")
                    nc.vector.reciprocal(rs[:], ssum[:])
                    nc.vector.tensor_scalar_mul(
                        ad_sb[:, k * T:(k + 1) * T], ex[:], rs[:])

            gates = {d: gpool.tile([128, NM * bl * T], BF16, name=f"gates_{d}")
                     for d in "fb"}
            hstore = {d: spool.tile([128, (T + 1) * B8], BF16, name=f"hstore_{d}")
                      for d in "fb"}
            cstate = {d: spool.tile([128, B8], F32, name=f"cstate_{d}") for d in "fb"}
            for d in "fb":
                z0 = 0 if d == "f" else T
                nc.vector.memset(hstore[d][:, z0 * B8:(z0 + 1) * B8], 0.0)
                nc.vector.memset(cstate[d][:], 0.0)

            # ---------------- Phase A: feedforward per sample ----------------
            with (
                tc.tile_pool(name="xb", bufs=2) as xbp,
                tc.tile_pool(name="mats", bufs=2) as mpool,
                tc.tile_pool(name="small", bufs=4) as smpool,
                tc.tile_pool(name="ps", bufs=4, space="PSUM") as pspool,
            ):
                with tc.For_i(0, bl, 1) as s:
                    x_bf = xbp.tile([128, KT, D], BF16, tag="x_bf")
                    if x_int8:
                        x8 = xbp.tile([128, KT, D], I8, tag="x8")
                        nc.sync.dma_start(
                            x8[:].rearrange("p (o k) d -> p o k d", o=1),
                            x_in[ds(s, 1)].rearrange("o (k p) d -> p o k d", p=128))
                        nc.vector.tensor_copy(x_bf[:], x8[:])
                    else:
                        nc.sync.dma_start(
                            x_bf[:].rearrange("p (o k) d -> p o k d", o=1),
                            x_in[ds(s, 1)].rearrange("o (k p) d -> p o k d", p=128))
                    xn_bf = xbp.tile([128, KT, D], BF16, tag="xn_bf")
                    xnt_bf = xbp.tile([128, KD, T], BF16, tag="xnt_bf")
                    dump = smpool.tile([128, D], BF16, tag="dump")
                    for k in range(KT):
                        ss = smpool.tile([128, 1], F32, tag="ss")
                        nc.scalar.activation(dump[:], x_bf[:, k, :], AF.Square,
                                             accum_out=ss[:])
                        sn = smpool.tile([128, 1], F32, tag="sn")
                        nc.scalar.activation(sn[:], ss[:], AF.Sqrt)
                        rn = smpool.tile([128, 1], F32, tag="rn")
                        nc.vector.reciprocal(rn[:], sn[:])
                        nc.vector.tensor_scalar_mul(xn_bf[:, k, :], x_bf[:, k, :], rn[:])
                    # Xn^T via DMA block transposes
                    for ti in range(KT):
                        for dj in range(KD):
                            nc.sync.dma_start_transpose(
                                xnt_bf[:, dj, ti * 128:(ti + 1) * 128],
                                xn_bf[:, ti, dj * 128:(dj + 1) * 128])
                    # A_S = Xn Xn^T   [T, T]
                    as_bf = mpool.tile([128, KT, T], BF16, tag="as_bf")
                    for m in range(KT):
                        ps = pspool.tile([128, T], F32, tag="psA")
                        for k in range(KD):
                            nc.tensor.matmul(
                                ps[:], xnt_bf[:, k, m * 128:(m + 1) * 128],
                                xnt_bf[:, k, :], start=(k == 0), stop=(k == KD - 1))
                        nc.vector.tensor_copy(as_bf[:, m, :], ps[:])
                    # P = A_D^T @ A_S
                    p_bf = mpool.tile([128, KT, T], BF16, tag="p_bf")
                    for m in range(KT):
                        ps = pspool.tile([128, T], F32, tag="psA")
                        for k in range(KT):
                            nc.tensor.matmul(
                                ps[:], ad_sb[:, k * T + m * 128:k * T + (m + 1) * 128],
                                as_bf[:, k, :], start=(k == 0), stop=(k == KT - 1))
                        nc.vector.tensor_copy(p_bf[:, m, :], ps[:])
                    # Xa^T = X^T @ P   [D, T]
                    xat_bf = mpool.tile([128, KD, T], BF16, tag="xat_bf")
                    for m in range(KD):
                        ps = pspool.tile([128, T], F32, tag="psA")
                        for k in range(KT):
                            nc.tensor.matmul(
                                ps[:], x_bf[:, k, m * 128:(m + 1) * 128],
                                p_bf[:, k, :], start=(k == 0), stop=(k == KT - 1))
                        nc.vector.tensor_copy(xat_bf[:, m, :], ps[:])
                    # gx^T = Wx^T @ Xa^T (+b) per direction
                    for di, d in enumerate("fb"):
                        for m in range(NM):
                            ps = pspool.tile([128, T], F32, tag="psA")
                            for k in range(KD):
                                nc.tensor.matmul(
                                    ps[:],
                                    wx_sb[d][:, k * G4 + m * 128:k * G4 + (m + 1) * 128],
                                    xat_bf[:, k, :], start=(k == 0), stop=(k == KD - 1))
                            nc.vector.tensor_scalar_add(
                                gates[d][:].rearrange(
                                    "p (t m s) -> p t m s",
                                    m=NM, s=bl)[:, :, m, ds(s, 1)],
                                ps[:].rearrange("p (t o) -> p t o", o=1),
                                bias_sb[:, di * NM + m:di * NM + m + 1])

            # ---------------- Phase R: BiLSTM recurrence (HW loop) ----------------
            with (
                tc.tile_pool(name="zps", bufs=4, space="PSUM") as zpool,
                tc.tile_pool(name="zsb", bufs=4) as zsbp,
                tc.tile_pool(name="sg", bufs=4) as sgp,
            ):
                with tc.For_i(0, T, 1) as i:
                    for d in "fb":
                        if d == "f":
                            roff = i * B8
                            woff = (i + 1) * B8
                            gcol = i * GB
                        else:
                            roff = (T - i) * B8
                            woff = (T - 1 - i) * B8
                            gcol = (T - 1 - i) * GB
                        hprev = sgp.tile([128, B8], BF16, tag=f"hprev_{d}")
                        nc.vector.tensor_copy(
                            hprev[:], hstore[d][:, ds(roff, B8)])
                        z_ps = zpool.tile([128, GB], F32, tag="z_ps")
                        for m in range(NM):
                            for j in range(KH):
                                nc.tensor.matmul(
                                    z_ps[:, m * bl:(m + 1) * bl],
                                    wh_sb[d][:, j * G4 + m * 128:j * G4 + (m + 1) * 128],
                                    hprev[:, j * bl:(j + 1) * bl],
                                    start=(j == 0), stop=(j == KH - 1))
                        z_sb = zsbp.tile([128, GB], F32, tag="z_sb")
                        nc.vector.scalar_tensor_tensor(
                            z_sb[:], z_ps[:], 1.0, gates[d][:, ds(gcol, GB)],
                            ALU.bypass, ALU.add)
                        sg = sgp.tile([128, GB], F32, tag="sg")
                        nc.scalar.activation(
                            sg[:, :6 * bl], z_sb[:, :6 * bl], AF.Sigmoid)
                        nc.scalar.activation(
                            sg[:, 6 * bl:], z_sb[:, 6 * bl:], AF.Tanh)
                        u = sgp.tile([128, B8], F32, tag="u")
                        nc.vector.scalar_tensor_tensor(
                            u[:], sg[:, :B8], 1.0, sg[:, 6 * bl:], ALU.bypass, ALU.mult)
                        q = sgp.tile([128, B8], F32, tag="q")
                        nc.vector.scalar_tensor_tensor(
                            q[:], sg[:, B8:2 * B8], 1.0, cstate[d][:],
                            ALU.bypass, ALU.mult)
                        nc.vector.scalar_tensor_tensor(
                            cstate[d][:], u[:], 1.0, q[:], ALU.bypass, ALU.add)
                        ct = sgp.tile([128, B8], F32, tag="ct")
                        nc.scalar.activation(ct[:], cstate[d][:], AF.Tanh)
                        nc.vector.scalar_tensor_tensor(
                            hstore[d][:, ds(woff, B8)],
                            sg[:, 2 * B8:3 * B8], 1.0, ct[:], ALU.bypass, ALU.mult)

            # ---------------- Phase C: classifier + softmax ----------------
            with (
                tc.tile_pool(name="cps", bufs=4, space="PSUM") as cpsp,
                tc.tile_pool(name="csb", bufs=4) as csbp,
            ):
                NTB = T // 128
                out_flat = out_d[:].rearrange("s t v -> (s t) v")
                # [p, c, t] views: c = within-slot column (j*bl + sample),
                # t = slot index (stride B8)
                vw = {d: hstore[d][:].rearrange("p (t c) -> p c t", c=B8)
                      for d in "fb"}
                with tc.For_i(0, bl, 1) as s:
                    for m in range(NTB):
                        hst = csbp.tile([128, 4, 128], BF16, tag="hst")
                        for k in range(4):
                            # fwd h(t) lives at slot t+1, bwd h(t) at slot t
                            d = "f" if k < 2 else "b"
                            t0 = m * 128 + (1 if k < 2 else 0)
                            nc.vector.tensor_copy(
                                hst[:, k:k + 1, :],
                                vw[d][:, ds((k % 2) * bl + s, 1), t0:t0 + 128])
                        ps = cpsp.tile([128, V + 1], F32, tag="psC")
                        for k in range(4):
                            nc.tensor.matmul(
                                ps[:], hst[:, k, :],
                                wc_sb[:, k * (V + 1):(k + 1) * (V + 1)],
                                start=(k == 0), stop=(k == 3))
                        lg = csbp.tile([128, V + 1], F32, tag="lg")
                        nc.vector.scalar_tensor_tensor(
                            lg[:], ps[:], 1.0, bc_sb[:], ALU.bypass, ALU.add)
                        e = csbp.tile([128, V + 1], F32, tag="e")
                        esum = csbp.tile([128, 1], F32, tag="esum")
                        nc.scalar.activation(e[:], lg[:], AF.Exp,
                                             accum_out=esum[:])
                        er = csbp.tile([128, 1], F32, tag="er")
                        nc.vector.reciprocal(er[:], esum[:])
                        o = csbp.tile([128, V + 1], BF16, tag="o")
                        nc.vector.tensor_scalar_mul(o[:], e[:], er[:])
                        nc.sync.dma_start(
                            out_flat[ds(s * T + m * 128, 128), :], o[:])

        _rep.close()

    nc.compile()
    return nc


def _quant_x(X):
    """X -> (shipped array, Wx scale).  int8 mode: global symmetric quant."""
    X = np.asarray(X, np.float32)
    if not X_INT8:
        return X.astype(BF16NP), 1.0
    g = max(float(np.abs(X).max()), 1e-30)
    q = np.multiply(X, 127.0 / g)
    np.rint(q, out=q)
    return q.astype(np.int8), g / 127.0


def _host_inputs(Wx_f, Wh_f, b_f, Wx_b, Wh_b, b_b, Wc, bc,
                 use_allgather=USE_ALLGATHER, wx_scale=1.0):
    com = {}
    bcrep = np.broadcast_to(np.asarray(bc, np.float32), (128, V + 1))
    wc_pack = np.zeros((128, VP), BF16NP)
    wc_pack[:, :4 * (V + 1)] = _host_stationary(
        np.asarray(Wc, np.float32)).astype(BF16NP)

    bias_cols = np.zeros((128, 2 * NM), np.float32)
    wx_pack = {}
    wh_pack = {}
    for di, (wx, wh, b) in enumerate(
            [(Wx_f, Wh_f, b_f), (Wx_b, Wh_b, b_b)]):
        d = "fb"[di]
        wxp = _permute_gates(np.asarray(wx, np.float32) * wx_scale)
        whp = _permute_gates(np.asarray(wh, np.float32))
        bp = _permute_gates(np.asarray(b, np.float32))
        wx_pack[d] = _host_stationary(wxp).astype(BF16NP)
        wh_pack[d] = _host_stationary(whp).astype(BF16NP)
        bias_cols[:, di * NM:(di + 1) * NM] = bp.reshape(NM, 128).T

    if use_allgather:
        bias_bf = np.ascontiguousarray(bias_cols).view(BF16NP)
        bc_bf = np.ascontiguousarray(bcrep).view(BF16NP)
        shards = []
        for r in range(NCORES):
            shards.append(np.concatenate([
                wx_pack["f"][:, r * WXS:(r + 1) * WXS],
                wx_pack["b"][:, r * WXS:(r + 1) * WXS],
                wh_pack["f"][:, r * WHS:(r + 1) * WHS],
                wh_pack["b"][:, r * WHS:(r + 1) * WHS],
                wc_pack[:, r * WCS:(r + 1) * WCS],
                np.zeros((128, 1), BF16NP), bias_bf, bc_bf,
            ], axis=1))
        com["_WSH_SHARDS"] = shards
    else:
        com["BIAS"] = bias_cols
        com["BCREP"] = bcrep.copy()
        com["WXf"] = wx_pack["f"]
        com["WXb"] = wx_pack["b"]
        com["WHf"] = wh_pack["f"]
        com["WHb"] = wh_pack["b"]
        com["WC"] = wc_pack
    return com


def _make_in_maps(com, Xq):
    shards = com.pop("_WSH_SHARDS", None)
    in_maps = []
    for i in range(NCORES):
        m = dict(com)
        m["XBF"] = Xq[i * BL:(i + 1) * BL]
        if shards is not None:
            m["PK"] = shards[i]
        in_maps.append(m)
    if shards is not None:
        com["_WSH_SHARDS"] = shards
    return in_maps


_CACHE = {}


def kernel(X, Wx_f, Wh_f, b_f, Wx_b, Wh_b, b_b, Wc, bc,
           label=None, inputlength=None, labellength=None):
    key = "prog"
    if key not in _CACHE:
        _CACHE[key] = build_program()
    nc = _CACHE[key]
    Xq, wxs = _quant_x(X)
    com = _host_inputs(Wx_f, Wh_f, b_f, Wx_b, Wh_b, b_b, Wc, bc, wx_scale=wxs)
    in_maps = _make_in_maps(com, Xq)
    res = bass_utils.run_bass_kernel_spmd(nc, in_maps, core_ids=list(range(NCORES)))
    out = np.concatenate([r["OUT"] for r in res.results], axis=0)
    return np.ascontiguousarray(out.astype(np.float32))


if __name__ == "__main__":
    import reference
    ins = {k: np.asarray(v) for k, v in reference.setup_inputs().items()}
    got = kernel(**ins)
    want = np.asarray(reference.reference(**ins))
    err = np.abs(got - want).max() / np.abs(want).max()
    print("abs-rel err:", err)


# revision 35
# speedup vs baseline: 1.0334x; 1.0334x over previous
"""Trainium2 Bass kernel: cosine-attention + positional-adjacency mix + BiLSTM + softmax classifier.

Model (per sample, reference semantics):
    Xn   = X / ||X||_row
    Xa   = (Xn Xn^T) @ A_D @ X          (A_D = row-normalized exp(-|i-j|/8), constant)
    h    = BiLSTM(Xa)                    (fwd + bwd, H=256)
    out  = softmax(h @ Wc + bc)

Strategy: data-parallel over batch (4 samples / core x 8 cores). All device
matmuls in bf16 with fp32 PSUM accumulation. The whole feedforward runs in
"transposed" layout so the LSTM gate math operates on 128-partition tiles:
    Xa^T = X^T @ (A_D^T @ (Xn Xn^T))     -- lhsT operands are natural-layout
    gx^T = Wx^T @ Xa^T  (+b)             -- [4H, T] per direction
LSTM recurrence keeps z^T tiles [128, 8*B]; gates are host-permuted to
[i, f, o, g] so ACT does one Sigmoid (i,f,o) + one Tanh (g) per step.
h is stored directly as bf16 in a (T+1)-slot ring ("hstore") whose slices are
the matmul moving operands of the next step -- no per-step transposes/copies.

Wall-clock-per-call optimizations (the metric is dominated by host<->device
transfer + per-call program-size-proportional overhead, NOT device execute
time, which measures as negligible):
  - X is shipped as globally-scaled int8 (8 MB instead of 32 MB f32). The
    scale cancels exactly in the cosine normalization and is folded into Wx
    on the host for the value path, so the device never sees it.
  - Weights are NOT replicated 8x: each core uploads a distinct 1/8 column
    shard of the packed weights (one "PK" blob per core, with the f32
    bias sections embedded via bitcast) and an on-device AllGather
    reconstructs the full set on every core.
  - The positional adjacency A_D is generated on device (iota/abs/exp).
  - The BiLSTM recurrence and the per-sample feedforward run inside tc.For_i
    hardware loops with ds() dynamic offsets: ~1k static instructions
    instead of ~24k, which cuts ~1s/call of per-call NEFF handling overhead.
  - Output is written as bf16 (3.2 MB) and widened to f32 on host.
  - The jax persistent compilation cache is enabled so repeat calls skip the
    XLA/neuronx recompile.
"""

import os
import numpy as np
import ml_dtypes

os.environ.setdefault("JAX_COMPILATION_CACHE_DIR", "/tmp/jaxcache")
try:
    import jax
    jax.config.update("jax_compilation_cache_dir",
                      os.environ["JAX_COMPILATION_CACHE_DIR"])
    jax.config.update("jax_persistent_cache_min_entry_size_bytes", -1)
    jax.config.update("jax_persistent_cache_min_compile_time_secs", 0)
except Exception:
    pass

import concourse.bass as bass
from concourse.bass import ds
import concourse.mybir as mybir
import concourse.bacc as bacc
import concourse.tile as tile
from concourse import bass_utils

F32 = mybir.dt.float32
BF16 = mybir.dt.bfloat16
AF = mybir.ActivationFunctionType
ALU = mybir.AluOpType
BF16NP = ml_dtypes.bfloat16

B_ALL, T_FULL, D, H, V = 32, 512, 512, 256, 96
SIGMA = 8.0
NCORES = 8
BL = B_ALL // NCORES          # samples per core
G4 = 4 * H                    # 1024 gate dims (permuted order i,f,o,g)
NM = G4 // 128                # 8 gate m-tiles
KD = D // 128                 # 4
KH = H // 128                 # 2

# packed-weight shard layout (columns per core): WXf | WXb | WHf | WHb | WCp
WXS = KD * G4 // NCORES       # 512
WHS = KH * G4 // NCORES       # 256
VP = 392                      # WC cols padded 388 -> 392 (divisible by 8)
WCS = VP // NCORES            # 49
SH = 2 * WXS + 2 * WHS + WCS  # 1585 shard cols
SHP = SH + 1                  # pad col so f32 sections sit at even offsets
PKC = SHP + 2 * (2 * NM) + 2 * (V + 1)  # + f32 BIAS/BCREP as raw bf16
USE_ALLGATHER = True
# Ship X as globally-scaled int8 (8 MB instead of 16 MB bf16). The global
# scale cancels exactly in the cosine normalization; for the value path it is
# folded into Wx on the host, so the device never sees it.
X_INT8 = True
I8 = mybir.dt.int8


def _host_stationary(q):
    """[R, C] -> [128, (R//128)*C]; k-th col-block = rows k*128:(k+1)*128."""
    r, c = q.shape
    return np.ascontiguousarray(
        q.reshape(r // 128, 128, c).transpose(1, 0, 2).reshape(128, (r // 128) * c)
    )


def _permute_gates(w):
    """Reorder last-dim gate blocks [i,f,g,o] -> [i,f,o,g]."""
    i, f, g, o = np.split(w, 4, axis=-1)
    return np.concatenate([i, f, o, g], axis=-1)


def build_program(t_param=T_FULL, n_devices=NCORES, bl=BL, reps=1,
                  use_allgather=USE_ALLGATHER, x_int8=X_INT8):
    T = t_param
    KT = T // 128
    nc = bacc.Bacc(
        "TRN2", target_bir_lowering=False, debug=False, enable_asserts=False,
        num_devices=n_devices,
    )

    x_in = nc.dram_tensor("XBF", [bl, T, D], I8 if x_int8 else BF16,
                          kind="ExternalInput")
    out_d = nc.dram_tensor("OUT", [bl, T, V + 1], BF16, kind="ExternalOutput")

    if use_allgather:
        pk_in = nc.dram_tensor("PK", [128, PKC], BF16, kind="ExternalInput")
        bias_in = pk_in[:, SHP:SHP + 4 * NM].bitcast(F32)
        bc_in = pk_in[:, SHP + 4 * NM:PKC].bitcast(F32)
        wbnc = nc.dram_tensor("WBNC", [128, SH], BF16)
        wall = nc.dram_tensor("WALL", [n_devices, 128, SH], BF16,
                              addr_space="Shared")
    else:
        bias_in = nc.dram_tensor("BIAS", [128, 2 * NM], F32,
                                 kind="ExternalInput")[:]
        bc_in = nc.dram_tensor("BCREP", [128, V + 1], F32,
                               kind="ExternalInput")[:]
        wx_in = {d: nc.dram_tensor(f"WX{d}", [128, KD * G4], BF16,
                                   kind="ExternalInput") for d in "fb"}
        wh_in = {d: nc.dram_tensor(f"WH{d}", [128, KH * G4], BF16,
                                   kind="ExternalInput") for d in "fb"}
        wc_in = nc.dram_tensor("WC", [128, VP], BF16, kind="ExternalInput")

    B8 = 2 * bl    # h-cols per hstore slot
    GB = NM * bl   # z free cols (8*B)

    from contextlib import ExitStack
    with tile.TileContext(nc) as tc:
        _rep = ExitStack()
        if reps > 1:
            _rep.enter_context(tc.For_i(0, reps, 1))
        with (
            tc.tile_pool(name="const", bufs=1) as cpool,
            tc.tile_pool(name="gates", bufs=1) as gpool,
            tc.tile_pool(name="state", bufs=1) as spool,
        ):
            if use_allgather:
                nc.sync.dma_start(wbnc[:], pk_in[:, :SH])
                nc.gpsimd.collective_compute(
                    "AllGather", ALU.bypass,
                    replica_groups=[list(range(n_devices))],
                    ins=[wbnc[:].opt()], outs=[wall[:].opt()])
                c0 = 0
                wsrc = {}
                for nm_, w_ in [("wxf", WXS), ("wxb", WXS), ("whf", WHS),
                                ("whb", WHS), ("wc", WCS)]:
                    wsrc[nm_] = wall[:, :, c0:c0 + w_].rearrange("r p c -> p r c")
                    c0 += w_
            wx_sb = {}
            wh_sb = {}
            for d in "fb":
                wx_sb[d] = cpool.tile([128, KD * G4], BF16, name=f"wx_{d}")
                nc.sync.dma_start(
                    wx_sb[d][:], wsrc[f"wx{d}"] if use_allgather else wx_in[d][:])
                wh_sb[d] = cpool.tile([128, KH * G4], BF16, name=f"wh_{d}")
                nc.sync.dma_start(
                    wh_sb[d][:], wsrc[f"wh{d}"] if use_allgather else wh_in[d][:])
            wc_sb = cpool.tile([128, VP], BF16)
            nc.sync.dma_start(wc_sb[:], wsrc["wc"] if use_allgather else wc_in[:])
            bias_sb = cpool.tile([128, 2 * NM], F32)
            nc.sync.dma_start(bias_sb[:], bias_in)
            bc_sb = cpool.tile([128, V + 1], F32)
            nc.sync.dma_start(bc_sb[:], bc_in)

            # A_D generated on device: ad_sb[p, k*T+c] = AD[k*128+p, c]
            ad_sb = cpool.tile([128, KT * T], BF16)
            with tc.tile_pool(name="adgen", bufs=2) as agp:
                for k in range(KT):
                    df = agp.tile([128, T], F32, tag="df")
                    nc.gpsimd.iota(df[:], pattern=[[1, T]], base=-(k * 128),
                                   channel_multiplier=-1,
                                   allow_small_or_imprecise_dtypes=True)
                    ab = agp.tile([128, T], F32, tag="ab")
                    nc.scalar.activation(ab[:], df[:], AF.Abs)
                    ex = agp.tile([128, T], F32, tag="ex")
                    ssum = agp.tile([128, 1], F32, tag="ssum")
                    nc.scalar.activation(ex[:], ab[:], AF.Exp,
                                         scale=-1.0 / SIGMA, accum_out=ssum[:])
                    rs = agp.tile([128, 1], F32, tag="rs")
                    nc.vector.reciprocal(rs[:], ssum[:])
                    nc.vector.tensor_scalar_mul(
                        ad_sb[:, k * T:(k + 1) * T], ex[:], rs[:])

            gates = {d: gpool.tile([128, NM * bl * T], BF16, name=f"gates_{d}")
                     for d in "fb"}
            hstore = {d: spool.tile([128, (T + 1) * B8], BF16, name=f"hstore_{d}")
                      for d in "fb"}
            cstate = {d: spool.tile([128, B8], F32, name=f"cstate_{d}") for d in "fb"}
            for d in "fb":
                z0 = 0 if d == "f" else T
                nc.vector.memset(hstore[d][:, z0 * B8:(z0 + 1) * B8], 0.0)
                nc.vector.memset(cstate[d][:], 0.0)

            # ---------------- Phase A: feedforward per sample ----------------
            with (
                tc.tile_pool(name="xb", bufs=2) as xbp,
                tc.tile_pool(name="mats", bufs=2) as mpool,
                tc.tile_pool(name="small", bufs=4) as smpool,
                tc.tile_pool(name="ps", bufs=4, space="PSUM") as pspool,
            ):
                with tc.For_i(0, bl, 1) as s:
                    x_bf = xbp.tile([128, KT, D], BF16, tag="x_bf")
                    if x_int8:
                        x8 = xbp.tile([128, KT, D], I8, tag="x8")
                        nc.sync.dma_start(
                            x8[:].rearrange("p (o k) d -> p o k d", o=1),
                            x_in[ds(s, 1)].rearrange("o (k p) d -> p o k d", p=128))
                        nc.vector.tensor_copy(x_bf[:], x8[:])
                    else:
                        nc.sync.dma_start(
                            x_bf[:].rearrange("p (o k) d -> p o k d", o=1),
                            x_in[ds(s, 1)].rearrange("o (k p) d -> p o k d", p=128))
                    xn_bf = xbp.tile([128, KT, D], BF16, tag="xn_bf")
                    xnt_bf = xbp.tile([128, KD, T], BF16, tag="xnt_bf")
                    dump = smpool.tile([128, D], BF16, tag="dump")
                    for k in range(KT):
                        ss = smpool.tile([128, 1], F32, tag="ss")
                        nc.scalar.activation(dump[:], x_bf[:, k, :], AF.Square,
                                             accum_out=ss[:])
                        sn = smpool.tile([128, 1], F32, tag="sn")
                        nc.scalar.activation(sn[:], ss[:], AF.Sqrt)
                        rn = smpool.tile([128, 1], F32, tag="rn")
                        nc.vector.reciprocal(rn[:], sn[:])
                        nc.vector.tensor_scalar_mul(xn_bf[:, k, :], x_bf[:, k, :], rn[:])
                    # Xn^T via DMA block transposes
                    for ti in range(KT):
                        for dj in range(KD):
                            nc.sync.dma_start_transpose(
                                xnt_bf[:, dj, ti * 128:(ti + 1) * 128],
                                xn_bf[:, ti, dj * 128:(dj + 1) * 128])
                    # A_S = Xn Xn^T   [T, T]
                    as_bf = mpool.tile([128, KT, T], BF16, tag="as_bf")
                    for m in range(KT):
                        ps = pspool.tile([128, T], F32, tag="psA")
                        for k in range(KD):
                            nc.tensor.matmul(
                                ps[:], xnt_bf[:, k, m * 128:(m + 1) * 128],
                                xnt_bf[:, k, :], start=(k == 0), stop=(k == KD - 1))
                        nc.vector.tensor_copy(as_bf[:, m, :], ps[:])
                    # P = A_D^T @ A_S
                    p_bf = mpool.tile([128, KT, T], BF16, tag="p_bf")
                    for m in range(KT):
                        ps = pspool.tile([128, T], F32, tag="psA")
                        for k in range(KT):
                            nc.tensor.matmul(
                                ps[:], ad_sb[:, k * T + m * 128:k * T + (m + 1) * 128],
                                as_bf[:, k, :], start=(k == 0), stop=(k == KT - 1))
                        nc.vector.tensor_copy(p_bf[:, m, :], ps[:])
                    # Xa^T = X^T @ P   [D, T]
                    xat_bf = mpool.tile([128, KD, T], BF16, tag="xat_bf")
                    for m in range(KD):
                        ps = pspool.tile([128, T], F32, tag="psA")
                        for k in range(KT):
                            nc.tensor.matmul(
                                ps[:], x_bf[:, k, m * 128:(m + 1) * 128],
                                p_bf[:, k, :], start=(k == 0), stop=(k == KT - 1))
                        nc.vector.tensor_copy(xat_bf[:, m, :], ps[:])
                    # gx^T = Wx^T @ Xa^T (+b) per direction
                    for di, d in enumerate("fb"):
                        for m in range(NM):
                            ps = pspool.tile([128, T], F32, tag="psA")
                            for k in range(KD):
                                nc.tensor.matmul(
                                    ps[:],
                                    wx_sb[d][:, k * G4 + m * 128:k * G4 + (m + 1) * 128],
                                    xat_bf[:, k, :], start=(k == 0), stop=(k == KD - 1))
                            nc.vector.tensor_scalar_add(
                                gates[d][:].rearrange(
                                    "p (t m s) -> p t m s",
                                    m=NM, s=bl)[:, :, m, ds(s, 1)],
                                ps[:].rearrange("p (t o) -> p t o", o=1),
                                bias_sb[:, di * NM + m:di * NM + m + 1])

            # ---------------- Phase R: BiLSTM recurrence (HW loop) ----------------
            with (
                tc.tile_pool(name="zps", bufs=4, space="PSUM") as zpool,
                tc.tile_pool(name="zsb", bufs=4) as zsbp,
                tc.tile_pool(name="sg", bufs=4) as sgp,
            ):
                with tc.For_i(0, T, 1) as i:
                    for d in "fb":
                        if d == "f":
                            roff = i * B8
                            woff = (i + 1) * B8
                            gcol = i * GB
                        else:
                            roff = (T - i) * B8
                            woff = (T - 1 - i) * B8
                            gcol = (T - 1 - i) * GB
                        hprev = sgp.tile([128, B8], BF16, tag=f"hprev_{d}")
                        nc.vector.tensor_copy(
                            hprev[:], hstore[d][:, ds(roff, B8)])
                        z_ps = zpool.tile([128, GB], F32, tag="z_ps")
                        for m in range(NM):
                            for j in range(KH):
                                nc.tensor.matmul(
                                    z_ps[:, m * bl:(m + 1) * bl],
                                    wh_sb[d][:, j * G4 + m * 128:j * G4 + (m + 1) * 128],
                                    hprev[:, j * bl:(j + 1) * bl],
                                    start=(j == 0), stop=(j == KH - 1))
                        z_sb = zsbp.tile([128, GB], F32, tag="z_sb")
                        nc.vector.scalar_tensor_tensor(
                            z_sb[:], z_ps[:], 1.0, gates[d][:, ds(gcol, GB)],
                            ALU.bypass, ALU.add)
                        sg = sgp.tile([128, GB], F32, tag="sg")
                        nc.scalar.activation(
                            sg[:, :6 * bl], z_sb[:, :6 * bl], AF.Sigmoid)
                        nc.scalar.activation(
                            sg[:, 6 * bl:], z_sb[:, 6 * bl:], AF.Tanh)
                        u = sgp.tile([128, B8], F32, tag="u")
                        nc.vector.scalar_tensor_tensor(
                            u[:], sg[:, :B8], 1.0, sg[:, 6 * bl:], ALU.bypass, ALU.mult)
                        q = sgp.tile([128, B8], F32, tag="q")
                        nc.vector.scalar_tensor_tensor(
                            q[:], sg[:, B8:2 * B8], 1.0, cstate[d][:],
                            ALU.bypass, ALU.mult)
                        nc.vector.scalar_tensor_tensor(
                            cstate[d][:], u[:], 1.0, q[:], ALU.bypass, ALU.add)
                        ct = sgp.tile([128, B8], F32, tag="ct")
                        nc.scalar.activation(ct[:], cstate[d][:], AF.Tanh)
                        nc.vector.scalar_tensor_tensor(
                            hstore[d][:, ds(woff, B8)],
                            sg[:, 2 * B8:3 * B8], 1.0, ct[:], ALU.bypass, ALU.mult)

            # ---------------- Phase C: classifier + softmax ----------------
            with (
                tc.tile_pool(name="cps", bufs=4, space="PSUM") as cpsp,
                tc.tile_pool(name="csb", bufs=4) as csbp,
            ):
                NTB = T // 128
                out_flat = out_d[:].rearrange("s t v -> (s t) v")
                # [p, c, t] views: c = within-slot column (j*bl + sample),
                # t = slot index (stride B8)
                vw = {d: hstore[d][:].rearrange("p (t c) -> p c t", c=B8)
                      for d in "fb"}
                with tc.For_i(0, bl, 1) as s:
                    for m in range(NTB):
                        hst = csbp.tile([128, 4, 128], BF16, tag="hst")
                        for k in range(4):
                            # fwd h(t) lives at slot t+1, bwd h(t) at slot t
                            d = "f" if k < 2 else "b"
                            t0 = m * 128 + (1 if k < 2 else 0)
                            nc.vector.tensor_copy(
                                hst[:, k:k + 1, :],
                                vw[d][:, ds((k % 2) * bl + s, 1), t0:t0 + 128])
                        ps = cpsp.tile([128, V + 1], F32, tag="psC")
                        for k in range(4):
                            nc.tensor.matmul(
                                ps[:], hst[:, k, :],
                                wc_sb[:, k * (V + 1):(k + 1) * (V + 1)],
                                start=(k == 0), stop=(k == 3))
                        lg = csbp.tile([128, V + 1], F32, tag="lg")
                        nc.vector.scalar_tensor_tensor(
                            lg[:], ps[:], 1.0, bc_sb[:], ALU.bypass, ALU.add)
                        e = csbp.tile([128, V + 1], F32, tag="e")
                        esum = csbp.tile([128, 1], F32, tag="esum")
                        nc.scalar.activation(e[:], lg[:], AF.Exp,
                                             accum_out=esum[:])
                        er = csbp.tile([128, 1], F32, tag="er")
                        nc.vector.reciprocal(er[:], esum[:])
                        o = csbp.tile([128, V + 1], BF16, tag="o")
                        nc.vector.tensor_scalar_mul(o[:], e[:], er[:])
                        nc.sync.dma_start(
                            out_flat[ds(s * T + m * 128, 128), :], o[:])

        _rep.close()

    nc.compile()
    return nc


_QBUF = {}


def _quant_x(X):
    """X -> (shipped array, Wx scale).  int8 mode: global symmetric quant."""
    X = np.asarray(X, np.float32)
    if not X_INT8:
        return X.astype(BF16NP), 1.0
    g = max(float(X.max()), -float(X.min()), 1e-30)
    if _QBUF.get("shape") != X.shape:
        _QBUF["shape"] = X.shape
        _QBUF["f"] = np.empty(X.shape, np.float32)
        _QBUF["i"] = np.empty(X.shape, np.int8)
    f, q = _QBUF["f"], _QBUF["i"]
    np.multiply(X, 127.0 / g, out=f)
    np.rint(f, out=f)
    np.copyto(q, f, casting="unsafe")
    return q, g / 127.0


def _host_inputs(Wx_f, Wh_f, b_f, Wx_b, Wh_b, b_b, Wc, bc,
                 use_allgather=USE_ALLGATHER, wx_scale=1.0):
    com = {}
    bcrep = np.broadcast_to(np.asarray(bc, np.float32), (128, V + 1))
    wc_pack = np.zeros((128, VP), BF16NP)
    wc_pack[:, :4 * (V + 1)] = _host_stationary(
        np.asarray(Wc, np.float32)).astype(BF16NP)

    bias_cols = np.zeros((128, 2 * NM), np.float32)
    wx_pack = {}
    wh_pack = {}
    for di, (wx, wh, b) in enumerate(
            [(Wx_f, Wh_f, b_f), (Wx_b, Wh_b, b_b)]):
        d = "fb"[di]
        wxp = _permute_gates(np.asarray(wx, np.float32) * wx_scale)
        whp = _permute_gates(np.asarray(wh, np.float32))
        bp = _permute_gates(np.asarray(b, np.float32))
        wx_pack[d] = _host_stationary(wxp).astype(BF16NP)
        wh_pack[d] = _host_stationary(whp).astype(BF16NP)
        bias_cols[:, di * NM:(di + 1) * NM] = bp.reshape(NM, 128).T

    if use_allgather:
        bias_bf = np.ascontiguousarray(bias_cols).view(BF16NP)
        bc_bf = np.ascontiguousarray(bcrep).view(BF16NP)
        shards = []
        for r in range(NCORES):
            shards.append(np.concatenate([
                wx_pack["f"][:, r * WXS:(r + 1) * WXS],
                wx_pack["b"][:, r * WXS:(r + 1) * WXS],
                wh_pack["f"][:, r * WHS:(r + 1) * WHS],
                wh_pack["b"][:, r * WHS:(r + 1) * WHS],
                wc_pack[:, r * WCS:(r + 1) * WCS],
                np.zeros((128, 1), BF16NP), bias_bf, bc_bf,
            ], axis=1))
        com["_WSH_SHARDS"] = shards
    else:
        com["BIAS"] = bias_cols
        com["BCREP"] = bcrep.copy()
        com["WXf"] = wx_pack["f"]
        com["WXb"] = wx_pack["b"]
        com["WHf"] = wh_pack["f"]
        com["WHb"] = wh_pack["b"]
        com["WC"] = wc_pack
    return com


def _make_in_maps(com, Xq):
    shards = com.pop("_WSH_SHARDS", None)
    in_maps = []
    for i in range(NCORES):
        m = dict(com)
        m["XBF"] = Xq[i * BL:(i + 1) * BL]
        if shards is not None:
            m["PK"] = shards[i]
        in_maps.append(m)
    if shards is not None:
        com["_WSH_SHARDS"] = shards
    return in_maps


_CACHE = {}


def kernel(X, Wx_f, Wh_f, b_f, Wx_b, Wh_b, b_b, Wc, bc,
           label=None, inputlength=None, labellength=None):
    key = "prog"
    if key not in _CACHE:
        _CACHE[key] = build_program()
    nc = _CACHE[key]
    Xq, wxs = _quant_x(X)
    com = _host_inputs(Wx_f, Wh_f, b_f, Wx_b, Wh_b, b_b, Wc, bc, wx_scale=wxs)
    in_maps = _make_in_maps(com, Xq)
    res = bass_utils.run_bass_kernel_spmd(nc, in_maps, core_ids=list(range(NCORES)))
    out = np.concatenate([r["OUT"] for r in res.results], axis=0)
    return np.ascontiguousarray(out.astype(np.float32))


if __name__ == "__main__":
    import reference
    ins = {k: np.asarray(v) for k, v in reference.setup_inputs().items()}
    got = kernel(**ins)
    want = np.asarray(reference.reference(**ins))
    err = np.abs(got - want).max() / np.abs(want).max()
    print("abs-rel err:", err)
